# revision 2
# baseline (speedup 1.0000x reference)
"""LIFSpike Trainium2 kernel (Bass/Tile), SPMD over 8 NeuronCores.

Reference semantics (T=4, tau=2, vth=1, vreset=0, decay_input=False,
detach_reset, hard reset):
    xs = x.reshape(T, B//T, C, H, W); v0 = 0
    h_t = 0.5 * v_t + x_t
    s_t = (h_t >= 1.0)
    v_{t+1} = h_t * (h_t < 1.0)
    out = s.reshape(B, C, H, W)

Kernel-side reformulation (exact in fp32 -- all rescalings are powers
of two, which commute with fp rounding):
    r_t := 2^t * h_t,  host supplies x'_t = 2^t * x_t
    r_0     = x'_0                        (DMA load)
    o_t     = sign(r_t - 2^t) -> int8     (ACT engine; exact: |r-2^t| is
                                           either 0 or >= ulp, so the
                                           fp32 subtract in ACT's affine
                                           stage has the exact sign, and
                                           Sign is a 1-ULP-budget table)
    q_t     = (r_t < 2^t) * r_t           (DVE STT, ping-pong tile)
    r_{t+1} = q_t + x'_{t+1}              (DVE tensor_tensor add)
    host decodes s_t = (o_t >= 0)         (+1/0 -> spike, -1 -> none)

Engine budget per core (measured-model, 0.96 GHz DVE / 1.2 GHz ACT):
  - DVE: 3 STT + 3 TT adds per chunk, fp32 1x mode -> ~55 us
  - ACT: 4 Sign ops per chunk -> ~32 us (plus store HWDGE triggers)
  - DMA: 16 MiB load + 4 MiB store @ ~358 GB/s HBM/NC -> ~58.6 us
  so the kernel is DMA-roofline-bound; the baseline (compares on DVE)
  was DVE-bound at ~71 us busy.
  - loads ride the SP HWDGE ring (nc.sync), stores the ACT ring
    (nc.scalar), so store waits never block next-iteration loads
  - x' chunk loads are emitted one timestep-section ahead
"""

import numpy as np

T = 4
BP = 32               # B // T
NCORES = 8
BPC = BP // NCORES    # chains per core = 4
SLICE = 256 * 32 * 32  # elements per (t, b) slice = 262144
P = 128
FREE_B = SLICE // P   # 2048
FREE_T = BPC * FREE_B  # 8192 free elements per timestep per core
FREE = T * FREE_T     # 32768

NCH = 4               # pipeline chunks
STORE_ENGINE = "scalar"
CMP = "act"           # "act": Sign on ACT -> int8; "dve": is_ge -> u8

_cache = {}


def build_program(reps=1, nch=NCH, cmp=CMP, store_engine=STORE_ENGINE,
                  state_bufs=1, out_bufs=2, xin_bufs=2, split_stores=True):
    import concourse.bass as bass
    import concourse.tile as tile
    from concourse import bacc, mybir

    Alu = mybir.AluOpType
    Act = mybir.ActivationFunctionType
    f32 = mybir.dt.float32
    odt = mybir.dt.int8 if cmp == "act" else mybir.dt.uint8
    F = FREE_T // nch

    nc = bacc.Bacc(debug=False)
    x = nc.dram_tensor("x", [P, FREE], f32, kind="ExternalInput").ap()
    s = nc.dram_tensor("s", [P, FREE], odt, kind="ExternalOutput").ap()

    st_eng = {"sync": nc.sync, "scalar": nc.scalar}[store_engine]

    with tile.TileContext(nc) as tc:
        with (
            tc.tile_pool(name="state", bufs=state_bufs) as rpool,
            tc.tile_pool(name="sout", bufs=out_bufs) as opool,
            tc.tile_pool(name="xin", bufs=xin_bufs) as xpool,
        ):
            for rep in range(reps):
                ra = [rpool.tile([P, F], f32, name=f"ra{c}")
                      for c in range(nch)]
                rb = [rpool.tile([P, F], f32, name=f"rb{c}")
                      for c in range(nch)]
                o = [opool.tile([P, T * F], odt, name=f"o{c}")
                     for c in range(nch)]
                xt = {}

                def emit_xin_loads(tr):
                    # x' loads for transition tr, emitted one section early
                    if tr > T - 2:
                        return
                    base = (tr + 1) * FREE_T
                    xtile = xpool.tile([P, FREE_T], f32, name="xw")
                    nc.sync.dma_start(xtile[:], x[:, base:base + FREE_T])
                    for c in range(nch):
                        xt[(tr, c)] = xtile[:, c * F:(c + 1) * F]

                for c in range(nch):
                    nc.sync.dma_start(ra[c][:], x[:, c * F:(c + 1) * F])
                emit_xin_loads(0)
                for t in range(T):
                    th = float(1 << t)
                    emit_xin_loads(t + 1)
                    cur = ra if t % 2 == 0 else rb
                    nxt = rb if t % 2 == 0 else ra
                    for c in range(nch):
                        if cmp == "act":
                            nc.scalar.activation(
                                o[c][:, t * F:(t + 1) * F], cur[c][:],
                                Act.Sign, bias=-th,
                            )
                        else:
                            nc.vector.tensor_scalar(
                                o[c][:, t * F:(t + 1) * F], cur[c][:], th,
                                None, Alu.is_ge,
                            )
                        if t < T - 1:
                            nc.vector.scalar_tensor_tensor(
                                nxt[c][:], cur[c][:], th, cur[c][:],
                                Alu.is_lt, Alu.mult,
                            )
                            nc.vector.tensor_tensor(
                                nxt[c][:], nxt[c][:], xt[(t, c)], Alu.add
                            )
                    if split_stores and t == 1:
                        # first half of each chunk's output (t=0,1) is
                        # final now; draining it early de-bursts the
                        # store tail
                        for c in range(nch):
                            st_eng.dma_start(
                                s[:, c * T * F:c * T * F + 2 * F],
                                o[c][:, 0:2 * F],
                            )
                    if t == T - 1:
                        for c in range(nch):
                            if split_stores:
                                st_eng.dma_start(
                                    s[:, c * T * F + 2 * F:(c + 1) * T * F],
                                    o[c][:, 2 * F:T * F],
                                )
                            else:
                                st_eng.dma_start(
                                    s[:, c * T * F:(c + 1) * T * F], o[c][:]
                                )
    nc.compile()
    return nc


def _shard(x):
    # x: (128, 256, 32, 32) f32 -> 8 per-core [128, 32768] f32 arrays,
    # timestep t pre-scaled by 2^t (exact in fp32); layout
    # x_core[p, t*8192 + b*2048 + j] = 2^t * x[t*32 + core*4 + b, flat]
    xr = np.ascontiguousarray(x).reshape(T, BP, SLICE)
    tscale = (2.0 ** np.arange(T, dtype=np.float32)).astype(np.float32)
    shards = []
    for k in range(NCORES):
        xk = xr[:, k * BPC:(k + 1) * BPC, :].reshape(T, BPC, P, FREE_B)
        xk = xk * tscale[:, None, None, None]
        xk = xk.transpose(2, 0, 1, 3).reshape(P, FREE)
        shards.append(np.asarray(xk, dtype=np.float32))
    return shards


def _unshard(parts, nch=NCH, cmp=CMP):
    # parts: 8 per-core [128, 32768] arrays, chunk-major layout
    # s[p, c*T*F + t*F + j] -> (128,256,32,32) f32.
    # cmp="act": values are int8 sign(r - th) in {-1, 0, +1}; spike iff >= 0
    # cmp="dve": values are u8 (r >= th) in {0, 1}
    F = FREE_T // nch
    cpb = nch // BPC  # chunks per chain
    out = np.empty((T, BP, SLICE), dtype=np.float32)
    for k, sk in enumerate(parts):
        sk = np.asarray(sk)
        if cmp == "act":
            sk = (sk.astype(np.int8) >= 0).astype(np.float32)
        else:
            sk = sk.astype(np.float32)
        sk = sk.reshape(P, BPC, cpb, T, F)
        out[:, k * BPC:(k + 1) * BPC, :] = (
            sk.transpose(3, 1, 0, 2, 4).reshape(T, BPC, SLICE)
        )
    return out.reshape(T * BP, 256, 32, 32)


def kernel(x):
    from concourse.bass_utils import run_bass_kernel_spmd

    if "nc" not in _cache:
        _cache["nc"] = build_program()
    nc = _cache["nc"]

    shards = _shard(np.asarray(x, dtype=np.float32))
    in_maps = [{"x": sk} for sk in shards]
    res = run_bass_kernel_spmd(nc, in_maps, list(range(NCORES)))
    return _unshard([res.results[k]["s"] for k in range(NCORES)])


# revision 8
# speedup vs baseline: 1.1229x; 1.1229x over previous
"""LIFSpike Trainium2 kernel (Bass/Tile), SPMD over 8 NeuronCores.

Reference semantics (T=4, tau=2, vth=1, vreset=0, decay_input=False,
detach_reset, hard reset):
    xs = x.reshape(T, B//T, C, H, W); v0 = 0
    h_t = 0.5 * v_t + x_t
    s_t = (h_t >= 1.0)
    v_{t+1} = h_t * (h_t < 1.0)
    out = s.reshape(B, C, H, W)

Kernel-side reformulation (exact in fp32 -- all rescalings are powers
of two, which commute with fp rounding):
    r_t := 2^t * h_t,  host supplies x'_t = 2^t * x_t
    r_0     = x'_0                        (DMA load)
    o_t     = sign(r_t - 2^t) -> int8     (ACT engine; exact: |r-2^t| is
                                           either 0 or >= ulp, so the
                                           fp32 subtract in ACT's affine
                                           stage has the exact sign, and
                                           Sign is a 1-ULP-budget table)
    q_t     = (r_t < 2^t) * r_t           (DVE STT, ping-pong tile)
    r_{t+1} = q_t + x'_{t+1}              (DVE tensor_tensor add)
    host decodes s_t = (o_t >= 0)         (+1/0 -> spike, -1 -> none)

Engine budget per core (measured-model, 0.96 GHz DVE / 1.2 GHz ACT):
  - DVE: 3 STT + 3 TT adds per chunk, fp32 1x mode -> ~55 us
  - ACT: 4 Sign ops per chunk -> ~32 us (plus store HWDGE triggers)
  - DMA: 16 MiB load + 4 MiB store @ ~358 GB/s HBM/NC -> ~58.6 us
  so the kernel is DMA-roofline-bound; the baseline (compares on DVE)
  was DVE-bound at ~71 us busy.
  - loads ride the SP HWDGE ring (nc.sync), stores the ACT ring
    (nc.scalar), so store waits never block next-iteration loads
  - x' chunk loads are emitted one timestep-section ahead
"""

import numpy as np

T = 4
BP = 32               # B // T
NCORES = 8
BPC = BP // NCORES    # chains per core = 4
SLICE = 256 * 32 * 32  # elements per (t, b) slice = 262144
P = 128
FREE_B = SLICE // P   # 2048
FREE_T = BPC * FREE_B  # 8192 free elements per timestep per core
FREE = T * FREE_T     # 32768

NCH = 2               # pipeline chunks
STORE_ENGINE = "scalar"
CMP = "act"           # "act": Sign on ACT -> int8; "dve": is_ge -> u8

_cache = {}


def build_program(reps=1, nch=NCH, cmp=CMP, store_engine=STORE_ENGINE,
                  state_bufs=1, out_bufs=2, xin_bufs=2, split_stores=True,
                  xin_wide=True):
    import concourse.bass as bass
    import concourse.tile as tile
    from concourse import bacc, mybir

    Alu = mybir.AluOpType
    Act = mybir.ActivationFunctionType
    f32 = mybir.dt.float32
    odt = mybir.dt.int8 if cmp == "act" else mybir.dt.uint8
    F = FREE_T // nch

    nc = bacc.Bacc(debug=False)
    x = nc.dram_tensor("x", [P, FREE], f32, kind="ExternalInput").ap()
    s = nc.dram_tensor("s", [P, FREE], odt, kind="ExternalOutput").ap()

    st_eng = {"sync": nc.sync, "scalar": nc.scalar}[store_engine]

    with tile.TileContext(nc) as tc:
        with (
            tc.tile_pool(name="state", bufs=state_bufs) as rpool,
            tc.tile_pool(name="sout", bufs=out_bufs) as opool,
            tc.tile_pool(name="xin", bufs=xin_bufs) as xpool,
        ):
            for rep in range(reps):
                ra = [rpool.tile([P, F], f32, name=f"ra{c}")
                      for c in range(nch)]
                rb = [rpool.tile([P, F], f32, name=f"rb{c}")
                      for c in range(nch)]
                o = [opool.tile([P, T * F], odt, name=f"o{c}")
                     for c in range(nch)]
                xt = {}

                def emit_xin_loads(tr):
                    # x' loads for transition tr, emitted one section early
                    if tr > T - 2:
                        return
                    base = (tr + 1) * FREE_T
                    if xin_wide:
                        xtile = xpool.tile([P, FREE_T], f32, name="xw")
                        nc.sync.dma_start(xtile[:], x[:, base:base + FREE_T])
                        for c in range(nch):
                            xt[(tr, c)] = xtile[:, c * F:(c + 1) * F]
                    else:
                        for c in range(nch):
                            xtile = xpool.tile([P, F], f32, name=f"xn{c}")
                            nc.sync.dma_start(
                                xtile[:], x[:, base + c * F:base + (c + 1) * F]
                            )
                            xt[(tr, c)] = xtile[:]

                for c in range(nch):
                    nc.sync.dma_start(ra[c][:], x[:, c * F:(c + 1) * F])
                emit_xin_loads(0)
                for t in range(T):
                    th = float(1 << t)
                    emit_xin_loads(t + 1)
                    cur = ra if t % 2 == 0 else rb
                    nxt = rb if t % 2 == 0 else ra
                    for c in range(nch):
                        if cmp == "act":
                            # sign(1 - r * 2^-t): +1 no spike, 0/-1 spike.
                            # bias=1.0 is a pre-registered const AP; the
                            # fma is single-rounded so the sign is exact.
                            nc.scalar.activation(
                                o[c][:, t * F:(t + 1) * F], cur[c][:],
                                Act.Sign, bias=1.0, scale=-1.0 / th,
                            )
                        else:
                            nc.vector.tensor_scalar(
                                o[c][:, t * F:(t + 1) * F], cur[c][:], th,
                                None, Alu.is_ge,
                            )
                        if t < T - 1:
                            nc.vector.scalar_tensor_tensor(
                                nxt[c][:], cur[c][:], th, cur[c][:],
                                Alu.is_lt, Alu.mult,
                            )
                            nc.vector.tensor_tensor(
                                nxt[c][:], nxt[c][:], xt[(t, c)], Alu.add
                            )
                    if split_stores and t == 1:
                        # first half of each chunk's output (t=0,1) is
                        # final now; draining it early de-bursts the
                        # store tail
                        for c in range(nch):
                            st_eng.dma_start(
                                s[:, c * T * F:c * T * F + 2 * F],
                                o[c][:, 0:2 * F],
                            )
                    if t == T - 1:
                        for c in range(nch):
                            if split_stores:
                                st_eng.dma_start(
                                    s[:, c * T * F + 2 * F:(c + 1) * T * F],
                                    o[c][:, 2 * F:T * F],
                                )
                            else:
                                st_eng.dma_start(
                                    s[:, c * T * F:(c + 1) * T * F], o[c][:]
                                )
    nc.compile()
    return nc


def _shard(x):
    # x: (128, 256, 32, 32) f32 -> 8 per-core [128, 32768] f32 arrays,
    # timestep t pre-scaled by 2^t (exact in fp32); layout
    # x_core[p, t*8192 + b*2048 + j] = 2^t * x[t*32 + core*4 + b, flat]
    xr = np.ascontiguousarray(x).reshape(T, BP, SLICE)
    tscale = (2.0 ** np.arange(T, dtype=np.float32)).astype(np.float32)
    shards = []
    for k in range(NCORES):
        xk = xr[:, k * BPC:(k + 1) * BPC, :].reshape(T, BPC, P, FREE_B)
        xk = xk * tscale[:, None, None, None]
        xk = xk.transpose(2, 0, 1, 3).reshape(P, FREE)
        shards.append(np.asarray(xk, dtype=np.float32))
    return shards


def _unshard(parts, nch=NCH, cmp=CMP):
    # parts: 8 per-core [128, 32768] arrays, chunk-major layout
    # s[p, c*T*F + t*F + j] -> (128,256,32,32) f32.
    # cmp="act": values are int8 sign(r - th) in {-1, 0, +1}; spike iff >= 0
    # cmp="dve": values are u8 (r >= th) in {0, 1}
    F = FREE_T // nch
    out = np.empty((T, BP, SLICE), dtype=np.float32)
    for k, sk in enumerate(parts):
        sk = np.asarray(sk)
        if cmp == "act":
            sk = (sk.astype(np.int8) <= 0).astype(np.float32)
        else:
            sk = sk.astype(np.float32)
        # storage: sk[p, c*T*F + t*F + jj]; concatenating chunks in order
        # walks the per-timestep free axis (b*FREE_B + j) contiguously
        sk = (
            sk.reshape(P, nch, T, F)
            .transpose(2, 1, 3, 0)            # [t, c, jj, p]
            .reshape(T, BPC, FREE_B, P)       # (c, jj) -> (b, j)
            .transpose(0, 1, 3, 2)            # [t, b, p, j]
        )
        out[:, k * BPC:(k + 1) * BPC, :] = sk.reshape(T, BPC, SLICE)
    return out.reshape(T * BP, 256, 32, 32)


def kernel(x):
    from concourse.bass_utils import run_bass_kernel_spmd

    if "nc" not in _cache:
        _cache["nc"] = build_program()
    nc = _cache["nc"]

    shards = _shard(np.asarray(x, dtype=np.float32))
    in_maps = [{"x": sk} for sk in shards]
    res = run_bass_kernel_spmd(nc, in_maps, list(range(NCORES)))
    return _unshard([res.results[k]["s"] for k in range(NCORES)])


# revision 12
# speedup vs baseline: 1.4991x; 1.3350x over previous
"""LIFSpike Trainium2 kernel (Bass/Tile), SPMD over 8 NeuronCores.

Reference semantics (T=4, tau=2, vth=1, vreset=0, decay_input=False,
detach_reset, hard reset):
    xs = x.reshape(T, B//T, C, H, W); v0 = 0
    h_t = 0.5 * v_t + x_t
    s_t = (h_t >= 1.0)
    v_{t+1} = h_t * (h_t < 1.0)
    out = s.reshape(B, C, H, W)

Kernel-side reformulation (exact in fp32 -- all rescalings are powers
of two, which commute with fp rounding):
    r_t := 2^t * h_t,  host supplies x'_t = 2^t * x_t
    r_0     = x'_0                        (DMA load)
    o_t     = sign(r_t - 2^t) -> int8     (ACT engine; exact: |r-2^t| is
                                           either 0 or >= ulp, so the
                                           fp32 subtract in ACT's affine
                                           stage has the exact sign, and
                                           Sign is a 1-ULP-budget table)
    q_t     = (r_t < 2^t) * r_t           (DVE STT, ping-pong tile)
    r_{t+1} = q_t + x'_{t+1}              (DVE tensor_tensor add)
    host decodes s_t = (o_t >= 0)         (+1/0 -> spike, -1 -> none)

Engine budget per core (measured-model, 0.96 GHz DVE / 1.2 GHz ACT):
  - DVE: 3 STT + 3 TT adds per chunk, fp32 1x mode -> ~55 us
  - ACT: 4 Sign ops per chunk -> ~32 us (plus store HWDGE triggers)
  - DMA: 16 MiB load + 4 MiB store @ ~358 GB/s HBM/NC -> ~58.6 us
  so the kernel is DMA-roofline-bound; the baseline (compares on DVE)
  was DVE-bound at ~71 us busy.
  - loads ride the SP HWDGE ring (nc.sync), stores the ACT ring
    (nc.scalar), so store waits never block next-iteration loads
  - x' chunk loads are emitted one timestep-section ahead
"""

import numpy as np

T = 4
BP = 32               # B // T
NCORES = 8
BPC = BP // NCORES    # chains per core = 4
SLICE = 256 * 32 * 32  # elements per (t, b) slice = 262144
P = 128
FREE_B = SLICE // P   # 2048
FREE_T = BPC * FREE_B  # 8192 free elements per timestep per core
FREE = T * FREE_T     # 32768

NCH = 2               # pipeline chunks
STORE_ENGINE = "scalar"
CMP = "act"           # "act": Sign on ACT -> int8; "dve": is_ge -> u8

_cache = {}


def build_program(reps=1, nch=NCH, cmp=CMP, store_engine=STORE_ENGINE,
                  state_bufs=1, out_bufs=2, xin_bufs=2, split_stores=True,
                  xin_wide=True, pack=False, pout_bufs=1, psum_bufs=4):
    import concourse.bass as bass
    import concourse.tile as tile
    from concourse import bacc, mybir

    Alu = mybir.AluOpType
    Act = mybir.ActivationFunctionType
    f32 = mybir.dt.float32
    bf16 = mybir.dt.bfloat16
    f8 = mybir.dt.float8e4
    i8 = mybir.dt.int8
    odt = mybir.dt.int8 if cmp == "act" else mybir.dt.uint8
    F = FREE_T // nch
    MG = 512              # matmul moving free dim / PSUM bank width

    nc = bacc.Bacc(debug=False)
    x = nc.dram_tensor("x", [P, FREE], f32, kind="ExternalInput").ap()
    if pack:
        assert cmp == "act"
        s = nc.dram_tensor("s", [P, FREE_T], i8, kind="ExternalOutput").ap()
        # w: four horizontal blocks w_t * I_128 (bf16), t-th block packs
        # the t-th sign plane with balanced-ternary weight {1,3,9,27}
        w = nc.dram_tensor("w", [P, 4 * P], bf16, kind="ExternalInput").ap()
    else:
        s = nc.dram_tensor("s", [P, FREE], odt, kind="ExternalOutput").ap()

    st_eng = {"sync": nc.sync, "scalar": nc.scalar}[store_engine]

    with tile.TileContext(nc) as tc:
        with (
            tc.tile_pool(name="state", bufs=state_bufs) as rpool,
            tc.tile_pool(name="sout", bufs=out_bufs) as opool,
            tc.tile_pool(name="xin", bufs=xin_bufs) as xpool,
            tc.tile_pool(name="pout", bufs=pout_bufs) as ppool,
            tc.psum_pool(name="acc", bufs=psum_bufs) as psum,
        ):
            if pack:
                wt = xpool.tile([P, 4 * P], bf16, name="wt")
                nc.sync.dma_start(wt[:], w[:])
            for rep in range(reps):
                ra = [rpool.tile([P, F], f32, name=f"ra{c}")
                      for c in range(nch)]
                rb = [rpool.tile([P, F], f32, name=f"rb{c}")
                      for c in range(nch)]
                if pack:
                    o = [opool.tile([P, T * F], f8, name=f"o{c}")
                         for c in range(nch)]
                    po = [ppool.tile([P, F], i8, name=f"p{c}")
                          for c in range(nch)]
                else:
                    o = [opool.tile([P, T * F], odt, name=f"o{c}")
                         for c in range(nch)]
                xt = {}

                def emit_xin_loads(tr):
                    # x' loads for transition tr, emitted one section early
                    if tr > T - 2:
                        return
                    base = (tr + 1) * FREE_T
                    if xin_wide:
                        xtile = xpool.tile([P, FREE_T], f32, name="xw")
                        nc.sync.dma_start(xtile[:], x[:, base:base + FREE_T])
                        for c in range(nch):
                            xt[(tr, c)] = xtile[:, c * F:(c + 1) * F]
                    else:
                        for c in range(nch):
                            xtile = xpool.tile([P, F], f32, name=f"xn{c}")
                            nc.sync.dma_start(
                                xtile[:], x[:, base + c * F:base + (c + 1) * F]
                            )
                            xt[(tr, c)] = xtile[:]

                for c in range(nch):
                    nc.sync.dma_start(ra[c][:], x[:, c * F:(c + 1) * F])
                emit_xin_loads(0)
                for t in range(T):
                    th = float(1 << t)
                    emit_xin_loads(t + 1)
                    cur = ra if t % 2 == 0 else rb
                    nxt = rb if t % 2 == 0 else ra
                    for c in range(nch):
                        if cmp == "act":
                            # sign(1 - r * 2^-t): +1 no spike, 0/-1 spike.
                            # bias=1.0 is a pre-registered const AP; the
                            # fma is single-rounded so the sign is exact.
                            nc.scalar.activation(
                                o[c][:, t * F:(t + 1) * F], cur[c][:],
                                Act.Sign, bias=1.0, scale=-1.0 / th,
                            )
                        else:
                            nc.vector.tensor_scalar(
                                o[c][:, t * F:(t + 1) * F], cur[c][:], th,
                                None, Alu.is_ge,
                            )
                        if t < T - 1:
                            nc.vector.scalar_tensor_tensor(
                                nxt[c][:], cur[c][:], th, cur[c][:],
                                Alu.is_lt, Alu.mult,
                            )
                            nc.vector.tensor_tensor(
                                nxt[c][:], nxt[c][:], xt[(t, c)], Alu.add
                            )
                    if not pack and split_stores and t == 1:
                        # first half of each chunk's output (t=0,1) is
                        # final now; draining it early de-bursts the
                        # store tail
                        for c in range(nch):
                            st_eng.dma_start(
                                s[:, c * T * F:c * T * F + 2 * F],
                                o[c][:, 0:2 * F],
                            )
                    if t == T - 1 and not pack:
                        for c in range(nch):
                            if split_stores:
                                st_eng.dma_start(
                                    s[:, c * T * F + 2 * F:(c + 1) * T * F],
                                    o[c][:, 2 * F:T * F],
                                )
                            else:
                                st_eng.dma_start(
                                    s[:, c * T * F:(c + 1) * T * F], o[c][:]
                                )
                    if t == T - 1 and pack:
                        # balanced-ternary pack: po = sum_t w_t * sign_t
                        # via 4 PE matmuls per 512-wide PSUM group, then
                        # ACT copies PSUM -> int8
                        for c in range(nch):
                            for g in range(F // MG):
                                pg = psum.tile([P, MG], f32, name="pg")
                                for tt in range(T):
                                    nc.tensor.matmul(
                                        pg[:],
                                        wt[:, tt * P:(tt + 1) * P],
                                        o[c][:, tt * F + g * MG:
                                             tt * F + (g + 1) * MG],
                                        start=(tt == 0), stop=(tt == T - 1),
                                    )
                                nc.scalar.activation(
                                    po[c][:, g * MG:(g + 1) * MG], pg[:],
                                    Act.Copy, bias=0.0, scale=1.0,
                                )
                            st_eng.dma_start(
                                s[:, c * F:(c + 1) * F], po[c][:]
                            )
    nc.compile()
    return nc


def _shard(x):
    # x: (128, 256, 32, 32) f32 -> 8 per-core [128, 32768] f32 arrays,
    # timestep t pre-scaled by 2^t (exact in fp32); layout
    # x_core[p, t*8192 + b*2048 + j] = 2^t * x[t*32 + core*4 + b, flat]
    xr = np.ascontiguousarray(x).reshape(T, BP, SLICE)
    tscale = (2.0 ** np.arange(T, dtype=np.float32)).astype(np.float32)
    shards = []
    for k in range(NCORES):
        xk = xr[:, k * BPC:(k + 1) * BPC, :].reshape(T, BPC, P, FREE_B)
        xk = xk * tscale[:, None, None, None]
        xk = xk.transpose(2, 0, 1, 3).reshape(P, FREE)
        shards.append(np.asarray(xk, dtype=np.float32))
    return shards


PACK_W = (1, 3, 9, 27)


def pack_weights():
    # [128, 4*128] bf16: block t = PACK_W[t] * I
    import ml_dtypes

    w = np.zeros((P, 4 * P), dtype=np.float32)
    for t in range(T):
        w[:, t * P:(t + 1) * P] = PACK_W[t] * np.eye(P, dtype=np.float32)
    return w.astype(ml_dtypes.bfloat16)


def _spike_luts():
    # p = sum_t PACK_W[t] * d_t with d_t in {-1, 0, +1}; spike iff d_t <= 0
    luts = np.zeros((T, 81), dtype=np.float32)
    for d0 in (-1, 0, 1):
        for d1 in (-1, 0, 1):
            for d2 in (-1, 0, 1):
                for d3 in (-1, 0, 1):
                    p = d0 + 3 * d1 + 9 * d2 + 27 * d3
                    for t, d in enumerate((d0, d1, d2, d3)):
                        luts[t, p + 40] = 1.0 if d <= 0 else 0.0
    return luts


def _unshard_packed(parts):
    luts = _spike_luts()
    out = np.empty((T, BP, SLICE), dtype=np.float32)
    for k, pk in enumerate(parts):
        idx = pk.astype(np.int16).reshape(P, BPC, FREE_B) + 40
        for t in range(T):
            out[t, k * BPC:(k + 1) * BPC, :] = (
                luts[t][idx].transpose(1, 0, 2).reshape(BPC, SLICE)
            )
    return out.reshape(T * BP, 256, 32, 32)


def _unshard(parts, nch=NCH, cmp=CMP):
    # parts: 8 per-core [128, 32768] arrays, chunk-major layout
    # s[p, c*T*F + t*F + j] -> (128,256,32,32) f32.
    # cmp="act": values are int8 sign(r - th) in {-1, 0, +1}; spike iff >= 0
    # cmp="dve": values are u8 (r >= th) in {0, 1}
    F = FREE_T // nch
    out = np.empty((T, BP, SLICE), dtype=np.float32)
    for k, sk in enumerate(parts):
        sk = np.asarray(sk)
        if cmp == "act":
            sk = (sk.astype(np.int8) <= 0).astype(np.float32)
        else:
            sk = sk.astype(np.float32)
        # storage: sk[p, c*T*F + t*F + jj]; concatenating chunks in order
        # walks the per-timestep free axis (b*FREE_B + j) contiguously
        sk = (
            sk.reshape(P, nch, T, F)
            .transpose(2, 1, 3, 0)            # [t, c, jj, p]
            .reshape(T, BPC, FREE_B, P)       # (c, jj) -> (b, j)
            .transpose(0, 1, 3, 2)            # [t, b, p, j]
        )
        out[:, k * BPC:(k + 1) * BPC, :] = sk.reshape(T, BPC, SLICE)
    return out.reshape(T * BP, 256, 32, 32)


PACK = False          # flipped after HW validation of the packed path


def kernel(x):
    from concourse.bass_utils import run_bass_kernel_spmd

    if "nc" not in _cache:
        _cache["nc"] = build_program(pack=PACK)
    nc = _cache["nc"]

    shards = _shard(np.asarray(x, dtype=np.float32))
    if PACK:
        wmat = pack_weights()
        in_maps = [{"x": sk, "w": wmat} for sk in shards]
    else:
        in_maps = [{"x": sk} for sk in shards]
    res = run_bass_kernel_spmd(nc, in_maps, list(range(NCORES)))
    parts = [res.results[k]["s"] for k in range(NCORES)]
    return _unshard_packed(parts) if PACK else _unshard(parts)
